# revision 1
# baseline (speedup 1.0000x reference)
"""CRF negative-log-likelihood loss (BERT_BiLSTM_CRF) on 8 TRN2 NeuronCores.

Strategy (data-parallel over batch, 64 sequences/core):
 - Linear-space forward algorithm with the 32x32 exp(transitions) matrix as
   PE matmul weights (block-diag [fwd | bwd]); per step one matmul + one DVE
   elementwise multiply by exp(emissions - MU).
 - Forward and backward (meet-in-the-middle) chains run in the same per-tick
   matmul, halving the serial step count to 1024.
 - Emissions are streamed in big DMA chunks, transposed to [tag, seq] layout
   on the PE, and exponentiated PSUM->SBUF on the scalar engine.
 - Periodic renorm (every 128 ticks) by a proxy-row reciprocal keeps fp32 in
   range; log(scale) accumulates into the per-sequence offset.
 - Gold-path score via two indirect (gathering) DMAs + free-dim reduces.
"""
import numpy as np

TAGSET = 32
START = 30
STOP = 31
B = 512
S = 2048
NCORES = 8
BC = B // NCORES          # 64 sequences per core
HALF = S // 2             # 1024 ticks per direction
CH = 64                   # emission steps per streamed chunk
NCH = HALF // CH          # 16 chunks per direction
MU = np.float32(4.3226)   # mean log-growth per step (measured offline)
REN = 128                 # renorm period in ticks

_CACHE = {}


def _build_nc(debug=False, gold=2):
    import concourse.bacc as bacc
    import concourse.bass as bass
    import concourse.tile as tile
    from concourse import mybir

    f32 = mybir.dt.float32
    i32 = mybir.dt.int32
    AF = mybir.ActivationFunctionType
    OP = mybir.AluOpType
    AX = mybir.AxisListType

    nc = bacc.Bacc("TRN2", target_bir_lowering=False, debug=False,
                   num_devices=NCORES)

    em_d = nc.dram_tensor("emissions", [BC, S, TAGSET], f32,
                          kind="ExternalInput").ap()
    tg_d = nc.dram_tensor("tags", [BC, S], i32, kind="ExternalInput").ap()
    tr_d = nc.dram_tensor("transitions", [TAGSET, TAGSET], f32,
                          kind="ExternalInput").ap()
    nll_d = nc.dram_tensor("nll", [1, BC], f32, kind="ExternalOutput").ap()
    dbg = {}
    if debug:
        for name in ["d_gold", "d_st0", "d_st128", "d_xt0", "d_w"]:
            dbg[name] = nc.dram_tensor(name, [64, 64], f32,
                                       kind="ExternalOutput").ap()

    with tile.TileContext(nc) as tc:
        with (
            tc.tile_pool(name="const", bufs=1) as cp,
            tc.tile_pool(name="chunk", bufs=3) as ccp,
            tc.tile_pool(name="oh", bufs=2) as ohp,
            tc.tile_pool(name="xt", bufs=6) as xtp,
            tc.tile_pool(name="state", bufs=3) as stp,
            tc.tile_pool(name="small", bufs=2) as smp,
            tc.tile_pool(name="trp", bufs=3, space="PSUM") as trp,
            tc.tile_pool(name="mmp", bufs=2, space="PSUM") as mmp,
            tc.tile_pool(name="finp", bufs=1, space="PSUM") as fip,
        ):
            # ---------------- setup: weights, identity, ones ----------------
            w = cp.tile([64, 64], f32)
            nc.vector.memset(w[:], 0.0)
            # fwd block: w[p, t] = trans[t, p]  (strided transpose DMA, tiny)
            nc.sync.dma_start(w[0:32, 0:32], tr_d.rearrange("a b -> b a"))
            # bwd block: w[32+p, 32+t] = trans[p, t]
            nc.sync.dma_start(w[32:64, 32:64], tr_d)
            # clamp -1e4 entries so the exp LUT stays in-range, then exp
            nc.vector.tensor_scalar_max(w[:], w[:], -80.0)
            nc.scalar.activation(w[:], w[:], AF.Exp)
            # zero the off-diagonal blocks again (exp(0)=1 crept in)
            nc.vector.memset(w[0:32, 32:64], 0.0)
            nc.vector.memset(w[32:64, 0:32], 0.0)

            ones_t = cp.tile([64, 64], f32)
            nc.vector.memset(ones_t[:], 1.0)
            negmu = cp.tile([64, 1], f32)
            nc.vector.memset(negmu[:], -float(MU))
            ident = cp.tile([64, 64], f32)
            nc.gpsimd.affine_select(
                out=ident[:], in_=ones_t[:], pattern=[[-1, 64]],
                compare_op=OP.is_equal, fill=0.0, base=0, channel_multiplier=1)

            # ---------------- gold emission score (one-hot) ----------------
            # e_score[b] = sum_s em[b, s, tags[b, s]]  computed per chunk as
            # sum((t-iota == tag) * em) with DVE is_equal + mult-reduce.
            # The transition part of the gold score is tiny (tags x 32x32
            # table) and is folded in on the host during unshard.
            tags_sb = cp.tile([BC, S], i32)
            nc.sync.dma_start(tags_sb[:], tg_d)
            iota_t = cp.tile([BC, CH * TAGSET], i32)
            nc.gpsimd.iota(iota_t[:], pattern=[[0, CH], [1, TAGSET]], base=0,
                           channel_multiplier=0)
            acc_e = cp.tile([BC, NCH], f32)
            nc.vector.memset(acc_e[:], 0.0)
            if debug:
                nc.sync.dma_start(dbg["d_gold"][:, 0:1], gold_c[:])

            # ---------------- scan state init ----------------
            offacc = cp.tile([64, 64], f32)
            nc.vector.memset(offacc[:], 0.0)

            state = stp.tile([64, 64], f32, tag="state")
            # one-hot inits: fwd rows = e_START, bwd rows = e_STOP
            nc.gpsimd.affine_select(
                out=state[0:32, :], in_=ones_t[0:32, :], pattern=[[0, 64]],
                compare_op=OP.is_equal, fill=0.0, base=-START,
                channel_multiplier=1)
            nc.gpsimd.affine_select(
                out=state[32:64, :], in_=ones_t[32:64, :], pattern=[[0, 64]],
                compare_op=OP.is_equal, fill=0.0, base=-STOP,
                channel_multiplier=1)

            # ---------------- main scan ----------------
            # Combined chunk layout per local step l (CH steps per chunk):
            #   cols [l*64, l*64+32)   = emissions[:, g*CH + l, :]  (fwd)
            #   cols [l*64+32, l*64+64) = emissions[:, S-1-g*CH-l, :]  (bwd)
            comb = None
            for tau in range(HALF):
                if tau % CH == 0:
                    g = tau // CH
                    comb = ccp.tile([BC, 2 * CH * TAGSET], f32, tag="comb")
                    cv = comb[:].rearrange("b (s u t) -> b s u t",
                                           u=2, t=TAGSET)
                    nc.sync.dma_start(cv[:, :, 0, :],
                                      em_d[:, g * CH:(g + 1) * CH, :])
                    nc.sync.dma_start(
                        cv[:, :, 1, :],
                        em_d[:, S - 1 - g * CH:S - (g + 1) * CH - 1:-1, :])
                    iview = iota_t[:].rearrange("b (l t) -> b l t", t=TAGSET)
                    if gold:
                        oh = ohp.tile([BC, 2 * CH * TAGSET], f32, tag="oh")
                        ov = oh[:].rearrange("b (l u t) -> b l u t",
                                             u=2, t=TAGSET)
                        for u in range(2):
                            if u == 0:
                                tsl = tags_sb[:, g * CH:(g + 1) * CH]
                            else:
                                tsl = tags_sb[:, S - 1 - g * CH:
                                              S - (g + 1) * CH - 1:-1]
                            tbc = tsl.rearrange("b l -> b l ()").to_broadcast(
                                [BC, CH, TAGSET])
                            nc.vector.tensor_tensor(
                                out=ov[:, :, u, :],
                                in0=iview, in1=tbc, op=OP.is_equal)
                        if gold >= 2:
                            scrap = ohp.tile([BC, 2 * CH * TAGSET], f32,
                                             tag="scrap")
                            nc.vector.tensor_mul(scrap[:], oh[:], comb[:])
                            nc.vector.tensor_reduce(
                                acc_e[:, g:g + 1], scrap[:],
                                axis=AX.X, op=OP.add)
                l = tau % CH

                tr_ps = trp.tile([64, 64], f32, tag="trps")
                nc.tensor.transpose(tr_ps[:], comb[:, l * 64:(l + 1) * 64],
                                    ident[:])
                xt = xtp.tile([64, 64], f32, tag="xt")
                nc.scalar.activation(xt[:], tr_ps[:], AF.Exp, bias=negmu[:])

                ps = mmp.tile([64, 64], f32, tag="mm")
                nc.tensor.matmul(ps[:], w[:], state[:], start=True, stop=True)
                nstate = stp.tile([64, 64], f32, tag="state")
                nc.vector.tensor_mul(nstate[:], ps[:], xt[:])
                state = nstate
                if debug and tau == 0:
                    nc.sync.dma_start(dbg["d_st0"], state[:])
                    nc.sync.dma_start(dbg["d_xt0"], xt[:])
                    nc.sync.dma_start(dbg["d_w"], w[:])
                if debug and tau == 130:
                    nc.sync.dma_start(dbg["d_st128"], state[:])

                if (tau + 1) % REN == 0:
                    rec = smp.tile([64, 64], f32, tag="rec")
                    nc.vector.reciprocal(rec[0:1, :], state[0:1, :])
                    nc.vector.reciprocal(rec[32:33, :], state[32:33, :])
                    bc_ps = fip.tile([64, 64], f32, tag="bc")
                    nc.tensor.matmul(bc_ps[0:32, :], ones_t[0:1, 0:32],
                                     rec[0:1, :], start=True, stop=True)
                    nc.tensor.matmul(bc_ps[32:64, :], ones_t[32:33, 0:32],
                                     rec[32:33, :], start=True, stop=True,
                                     tile_position=(32, 32))
                    lg = smp.tile([64, 64], f32, tag="lg")
                    nc.scalar.activation(lg[0:1, :], state[0:1, :], AF.Ln)
                    nc.scalar.activation(lg[32:33, :], state[32:33, :], AF.Ln)
                    nc.vector.tensor_add(offacc[0:1, :], offacc[0:1, :],
                                         lg[0:1, :])
                    nc.vector.tensor_add(offacc[32:33, :], offacc[32:33, :],
                                         lg[32:33, :])
                    rstate = stp.tile([64, 64], f32, tag="state")
                    nc.vector.tensor_mul(rstate[:], state[:], bc_ps[:])
                    state = rstate

            # ---------------- finale ----------------
            # beta_1023 = M^T gamma_1024: bwd-final matmul with weights
            # placed so the output lands on partitions 0-31 (aligned with
            # the fwd state for the elementwise dot).
            wb = cp.tile([64, 64], f32)
            nc.vector.memset(wb[:], 0.0)
            nc.sync.dma_start(wb[32:64, 0:32], w[32:64, 32:64])
            psf = mmp.tile([64, 64], f32, tag="mm")
            nc.tensor.matmul(psf[0:32, :], wb[32:64, 0:32], state[32:64, :],
                             start=True, stop=True)
            zp = smp.tile([64, 64], f32, tag="zp")
            nc.vector.tensor_mul(zp[0:32, :], psf[0:32, :], state[0:32, :])
            zsum = fip.tile([1, 64], f32, tag="zsum")
            nc.tensor.matmul(zsum[0:1, :], ones_t[0:32, 0:1], zp[0:32, :],
                             start=True, stop=True)
            gold_c = cp.tile([BC, 1], f32)
            nc.vector.tensor_reduce(gold_c[:], acc_e[:], axis=AX.X, op=OP.add)
            lz = smp.tile([64, 64], f32, tag="lz")
            nc.scalar.activation(lz[0:1, :], zsum[0:1, :], AF.Ln)
            ob = smp.tile([64, 64], f32, tag="ob")
            nc.sync.dma_start(ob[0:1, :], offacc[32:33, :])
            nc.vector.tensor_add(lz[0:1, :], lz[0:1, :], offacc[0:1, :])
            nc.vector.tensor_add(lz[0:1, :], lz[0:1, :], ob[0:1, :])
            # logZ = lz + MU*S;   nll = logZ - gold
            goldT = fip.tile([1, 64], f32, tag="goldT")
            nc.tensor.transpose(goldT[0:1, :], gold_c[:, 0:1], ident[:])
            nc.vector.tensor_sub(lz[0:1, :], lz[0:1, :], goldT[0:1, :])
            nc.vector.tensor_scalar_add(lz[0:1, :], lz[0:1, :],
                                        float(MU) * S)
            nc.sync.dma_start(nll_d, lz[0:1, :])

    nc.compile()
    return nc


def _get_nc():
    if "nc" not in _CACHE:
        _CACHE["nc"] = _build_nc()
    return _CACHE["nc"]


def kernel(emissions, transitions, tags):
    from concourse.bass_utils import run_bass_kernel_spmd

    em = np.ascontiguousarray(np.asarray(emissions, dtype=np.float32))
    tr = np.ascontiguousarray(np.asarray(transitions, dtype=np.float32))
    tg = np.ascontiguousarray(np.asarray(tags, dtype=np.int32))

    nc = _get_nc()
    in_maps = [
        {
            "emissions": em[c * BC:(c + 1) * BC],
            "tags": tg[c * BC:(c + 1) * BC],
            "transitions": tr,
        }
        for c in range(NCORES)
    ]
    res = run_bass_kernel_spmd(nc, in_maps, list(range(NCORES)))
    nll = np.concatenate([res.results[c]["nll"][0] for c in range(NCORES)])
    t_sc = (tr[tg[:, 1:], tg[:, :-1]].sum(axis=1)
            + tr[tg[:, 0], START] + tr[STOP, tg[:, -1]])
    total = np.sum(nll.astype(np.float64)) - np.sum(t_sc.astype(np.float64))
    return np.array(total, dtype=np.float32)



# revision 5
# speedup vs baseline: 67.6872x; 67.6872x over previous
"""CRF negative-log-likelihood loss (BERT_BiLSTM_CRF) on 8 TRN2 NeuronCores.

Strategy (data-parallel over batch, 64 sequences/core):
 - Linear-space forward algorithm with the 32x32 exp(transitions) matrix as
   PE matmul weights (block-diag [fwd | bwd]); per step one matmul + one DVE
   elementwise multiply by exp(emissions - MU).
 - Forward and backward (meet-in-the-middle) chains run in the same per-tick
   matmul, halving the serial step count to 1024.
 - Emissions are streamed in big DMA chunks, transposed to [tag, seq] layout
   on the PE, and exponentiated PSUM->SBUF on the scalar engine.
 - Periodic renorm (every 128 ticks) by a proxy-row reciprocal keeps fp32 in
   range; log(scale) accumulates into the per-sequence offset.
 - Gold-path score via two indirect (gathering) DMAs + free-dim reduces.
"""
import numpy as np

TAGSET = 32
START = 30
STOP = 31
B = 512
S = 2048
NCORES = 8
BC = B // NCORES          # 64 sequences per core
HALF = S // 2             # 1024 ticks per direction
CH = 64                   # emission steps per streamed chunk
NCH = HALF // CH          # 16 chunks per direction
MU = np.float32(4.3226)   # mean log-growth per step (measured offline)
REN = 128                 # renorm period in ticks

_CACHE = {}


def _build_nc(debug=False, gold=2, reps=1):
    import concourse.bacc as bacc
    import concourse.bass as bass
    import concourse.tile as tile
    from concourse import mybir

    f32 = mybir.dt.float32
    bf16 = mybir.dt.bfloat16
    i32 = mybir.dt.int32
    AF = mybir.ActivationFunctionType
    OP = mybir.AluOpType
    AX = mybir.AxisListType

    nc = bacc.Bacc("TRN2", target_bir_lowering=False, debug=False,
                   num_devices=NCORES)

    em_d = nc.dram_tensor("emissions", [BC, S, TAGSET], f32,
                          kind="ExternalInput").ap()
    tg_d = nc.dram_tensor("tags", [BC, S], i32, kind="ExternalInput").ap()
    tr_d = nc.dram_tensor("transitions", [TAGSET, TAGSET], f32,
                          kind="ExternalInput").ap()
    nll_d = nc.dram_tensor("nll", [1, BC], f32, kind="ExternalOutput").ap()
    dbg = {}
    if debug:
        for name in ["d_gold", "d_st0", "d_st128", "d_xt0", "d_w"]:
            dbg[name] = nc.dram_tensor(name, [64, 64], f32,
                                       kind="ExternalOutput").ap()

    with tile.TileContext(nc) as tc:
        with (
            tc.tile_pool(name="const", bufs=1) as cp,
            tc.tile_pool(name="chunk", bufs=3) as ccp,
            tc.tile_pool(name="oh", bufs=2) as ohp,
            tc.tile_pool(name="xt", bufs=6) as xtp,
            tc.tile_pool(name="state", bufs=3) as stp,
            tc.tile_pool(name="small", bufs=2) as smp,
            tc.tile_pool(name="trp", bufs=3, space="PSUM") as trp,
            tc.tile_pool(name="mmp", bufs=2, space="PSUM") as mmp,
            tc.tile_pool(name="finp", bufs=1, space="PSUM") as fip,
        ):
            # Optional on-device repetition (used only by test.py timing —
            # slope over reps cancels host/tunnel dispatch overhead).
            _loop = None
            if reps > 1:
                _loop = tc.For_i(
                    0, reps, 1,
                    hint_engines=(mybir.EngineType.PE, mybir.EngineType.DVE,
                                  mybir.EngineType.Activation))
                _loop.__enter__()

            # ---------------- setup: weights, identity, ones ----------------
            w = cp.tile([64, 64], f32)
            nc.vector.memset(w[:], 0.0)
            # fwd block: w[p, t] = trans[t, p]  (strided transpose DMA, tiny)
            nc.sync.dma_start(w[0:32, 0:32], tr_d.rearrange("a b -> b a"))
            # bwd block: w[32+p, 32+t] = trans[p, t]
            nc.sync.dma_start(w[32:64, 32:64], tr_d)
            # clamp -1e4 entries so the exp LUT stays in-range, then exp
            nc.vector.tensor_scalar_max(w[:], w[:], -80.0)
            nc.scalar.activation(w[:], w[:], AF.Exp)
            # zero the off-diagonal blocks again (exp(0)=1 crept in)
            nc.vector.memset(w[0:32, 32:64], 0.0)
            nc.vector.memset(w[32:64, 0:32], 0.0)

            ones_t = cp.tile([64, 64], f32)
            nc.vector.memset(ones_t[:], 1.0)
            negmu = cp.tile([64, 1], f32)
            nc.vector.memset(negmu[:], -float(MU))
            ident = cp.tile([64, 64], f32)
            nc.gpsimd.affine_select(
                out=ident[:], in_=ones_t[:], pattern=[[-1, 64]],
                compare_op=OP.is_equal, fill=0.0, base=0, channel_multiplier=1)

            # ---------------- gold emission score (one-hot) ----------------
            # e_score[b] = sum_s em[b, s, tags[b, s]]  computed per chunk as
            # sum((t-iota == tag) * em) with DVE is_equal + mult-reduce.
            # The transition part of the gold score is tiny (tags x 32x32
            # table) and is folded in on the host during unshard.
            tags_sb = cp.tile([BC, S], i32)
            nc.sync.dma_start(tags_sb[:], tg_d)
            iota_t = cp.tile([BC, CH * TAGSET], i32)
            nc.gpsimd.iota(iota_t[:], pattern=[[0, CH], [1, TAGSET]], base=0,
                           channel_multiplier=0)
            acc_e = cp.tile([BC, NCH], f32)
            nc.vector.memset(acc_e[:], 0.0)
            if debug:
                nc.sync.dma_start(dbg["d_gold"][:, 0:1], gold_c[:])

            # ---------------- scan state init ----------------
            offacc = cp.tile([64, 64], f32)
            nc.vector.memset(offacc[:], 0.0)

            state = stp.tile([64, 64], f32, tag="state")
            # one-hot inits: fwd rows = e_START, bwd rows = e_STOP
            nc.gpsimd.affine_select(
                out=state[0:32, :], in_=ones_t[0:32, :], pattern=[[0, 64]],
                compare_op=OP.is_equal, fill=0.0, base=-START,
                channel_multiplier=1)
            nc.gpsimd.affine_select(
                out=state[32:64, :], in_=ones_t[32:64, :], pattern=[[0, 64]],
                compare_op=OP.is_equal, fill=0.0, base=-STOP,
                channel_multiplier=1)

            # ---------------- main scan ----------------
            # Combined chunk layout per local step l (CH steps per chunk):
            #   cols [l*64, l*64+32)   = emissions[:, g*CH + l, :]  (fwd)
            #   cols [l*64+32, l*64+64) = emissions[:, S-1-g*CH-l, :]  (bwd)
            comb = None
            for tau in range(HALF):
                if tau % CH == 0:
                    g = tau // CH
                    comb = ccp.tile([BC, 2 * CH * TAGSET], f32, tag="comb")
                    cv = comb[:].rearrange("b (s u t) -> b s u t",
                                           u=2, t=TAGSET)
                    nc.sync.dma_start(cv[:, :, 0, :],
                                      em_d[:, g * CH:(g + 1) * CH, :])
                    nc.sync.dma_start(
                        cv[:, :, 1, :],
                        em_d[:, S - 1 - g * CH:S - (g + 1) * CH - 1:-1, :])
                    iview = iota_t[:].rearrange("b (l t) -> b l t", t=TAGSET)
                    if gold:
                        oh = ohp.tile([BC, 2 * CH * TAGSET], f32, tag="oh")
                        ov = oh[:].rearrange("b (l u t) -> b l u t",
                                             u=2, t=TAGSET)
                        for u in range(2):
                            if u == 0:
                                tsl = tags_sb[:, g * CH:(g + 1) * CH]
                            else:
                                tsl = tags_sb[:, S - 1 - g * CH:
                                              S - (g + 1) * CH - 1:-1]
                            tbc = tsl.rearrange("b l -> b l ()").to_broadcast(
                                [BC, CH, TAGSET])
                            nc.vector.tensor_tensor(
                                out=ov[:, :, u, :],
                                in0=iview, in1=tbc, op=OP.is_equal)
                        if gold >= 2:
                            scrap = ohp.tile([BC, 2 * CH * TAGSET], f32,
                                             tag="scrap")
                            nc.vector.tensor_mul(scrap[:], oh[:], comb[:])
                            nc.vector.tensor_reduce(
                                acc_e[:, g:g + 1], scrap[:],
                                axis=AX.X, op=OP.add)
                l = tau % CH

                tr_ps = trp.tile([64, 64], f32, tag="trps")
                nc.tensor.transpose(tr_ps[:], comb[:, l * 64:(l + 1) * 64],
                                    ident[:])
                xt = xtp.tile([64, 64], f32, tag="xt")
                nc.scalar.activation(xt[:], tr_ps[:], AF.Exp, bias=negmu[:])

                ps = mmp.tile([64, 64], f32, tag="mm")
                nc.tensor.matmul(ps[:], w[:], state[:], start=True, stop=True)
                nstate = stp.tile([64, 64], f32, tag="state")
                nc.vector.tensor_mul(nstate[:], ps[:], xt[:])
                state = nstate
                if debug and tau == 0:
                    nc.sync.dma_start(dbg["d_st0"], state[:])
                    nc.sync.dma_start(dbg["d_xt0"], xt[:])
                    nc.sync.dma_start(dbg["d_w"], w[:])
                if debug and tau == 130:
                    nc.sync.dma_start(dbg["d_st128"], state[:])

                if (tau + 1) % REN == 0:
                    rec = smp.tile([64, 64], f32, tag="rec")
                    nc.vector.reciprocal(rec[0:1, :], state[0:1, :])
                    nc.vector.reciprocal(rec[32:33, :], state[32:33, :])
                    bc_ps = fip.tile([64, 64], f32, tag="bc")
                    nc.tensor.matmul(bc_ps[0:32, :], ones_t[0:1, 0:32],
                                     rec[0:1, :], start=True, stop=True)
                    nc.tensor.matmul(bc_ps[32:64, :], ones_t[32:33, 0:32],
                                     rec[32:33, :], start=True, stop=True,
                                     tile_position=(32, 32))
                    lg = smp.tile([64, 64], f32, tag="lg")
                    nc.scalar.activation(lg[0:1, :], state[0:1, :], AF.Ln)
                    nc.scalar.activation(lg[32:33, :], state[32:33, :], AF.Ln)
                    nc.vector.tensor_add(offacc[0:1, :], offacc[0:1, :],
                                         lg[0:1, :])
                    nc.vector.tensor_add(offacc[32:33, :], offacc[32:33, :],
                                         lg[32:33, :])
                    rstate = stp.tile([64, 64], f32, tag="state")
                    nc.vector.tensor_mul(rstate[:], state[:], bc_ps[:])
                    state = rstate

            # ---------------- finale ----------------
            # beta_1023 = M^T gamma_1024: bwd-final matmul with weights
            # placed so the output lands on partitions 0-31 (aligned with
            # the fwd state for the elementwise dot).
            wb = cp.tile([64, 64], f32)
            nc.vector.memset(wb[:], 0.0)
            nc.sync.dma_start(wb[32:64, 0:32], w[32:64, 32:64])
            psf = mmp.tile([64, 64], f32, tag="mm")
            nc.tensor.matmul(psf[0:32, :], wb[32:64, 0:32], state[32:64, :],
                             start=True, stop=True)
            zp = smp.tile([64, 64], f32, tag="zp")
            nc.vector.tensor_mul(zp[0:32, :], psf[0:32, :], state[0:32, :])
            zsum = fip.tile([1, 64], f32, tag="zsum")
            nc.tensor.matmul(zsum[0:1, :], ones_t[0:32, 0:1], zp[0:32, :],
                             start=True, stop=True)
            gold_c = cp.tile([BC, 1], f32)
            nc.vector.tensor_reduce(gold_c[:], acc_e[:], axis=AX.X, op=OP.add)
            lz = smp.tile([64, 64], f32, tag="lz")
            nc.scalar.activation(lz[0:1, :], zsum[0:1, :], AF.Ln)
            ob = smp.tile([64, 64], f32, tag="ob")
            nc.sync.dma_start(ob[0:1, :], offacc[32:33, :])
            nc.vector.tensor_add(lz[0:1, :], lz[0:1, :], offacc[0:1, :])
            nc.vector.tensor_add(lz[0:1, :], lz[0:1, :], ob[0:1, :])
            # logZ = lz + MU*S;   nll = logZ - gold
            goldT = fip.tile([1, 64], f32, tag="goldT")
            nc.tensor.transpose(goldT[0:1, :], gold_c[:, 0:1], ident[:])
            nc.vector.tensor_sub(lz[0:1, :], lz[0:1, :], goldT[0:1, :])
            nc.vector.tensor_scalar_add(lz[0:1, :], lz[0:1, :],
                                        float(MU) * S)
            nc.sync.dma_start(nll_d, lz[0:1, :])

            if _loop is not None:
                _loop.__exit__(None, None, None)

    nc.compile()
    return nc


def _get_nc():
    if "nc" not in _CACHE:
        _CACHE["nc"] = _build_nc()
    return _CACHE["nc"]


def kernel(emissions, transitions, tags):
    from concourse.bass_utils import run_bass_kernel_spmd

    em = np.ascontiguousarray(np.asarray(emissions, dtype=np.float32))
    tr = np.ascontiguousarray(np.asarray(transitions, dtype=np.float32))
    tg = np.ascontiguousarray(np.asarray(tags, dtype=np.int32))

    nc = _get_nc()
    in_maps = [
        {
            "emissions": em[c * BC:(c + 1) * BC],
            "tags": tg[c * BC:(c + 1) * BC],
            "transitions": tr,
        }
        for c in range(NCORES)
    ]
    res = run_bass_kernel_spmd(nc, in_maps, list(range(NCORES)))
    nll = np.concatenate([res.results[c]["nll"][0] for c in range(NCORES)])
    t_sc = (tr[tg[:, 1:], tg[:, :-1]].sum(axis=1)
            + tr[tg[:, 0], START] + tr[STOP, tg[:, -1]])
    total = np.sum(nll.astype(np.float64)) - np.sum(t_sc.astype(np.float64))
    return np.array(total, dtype=np.float32)



# revision 17
# speedup vs baseline: 956.4339x; 14.1302x over previous
"""CRF negative-log-likelihood loss (BERT_BiLSTM_CRF) on 8 TRN2 NeuronCores.

Strategy (data-parallel over batch, 64 sequences/core):
 - Linear-space forward algorithm with the 32x32 exp(transitions) matrix as
   PE matmul weights (block-diag [fwd | bwd], bf16); per step one bf16 matmul
   + one DVE elementwise multiply by exp(emissions - MU).
 - Forward and backward (meet-in-the-middle) chains run in the same per-tick
   matmul, halving the serial step count to 1024.
 - The exp(emissions) pipeline is decoupled from the scan: per tick one PE
   transpose lands in a grouped PSUM tile, one batched ACT exp per 8 ticks
   writes a bf16 xt chunk a full 64 ticks ahead of use. The scan's serial
   critical path is only matmul -> DVE multiply -> matmul.
 - Emissions stream in big per-chunk DMAs (fwd natural, bwd reversed).
 - Periodic renorm (every 128 ticks) by a proxy-row reciprocal keeps bf16
   exponents in range; log(scale) accumulates into the per-sequence offset.
 - Gold emission score via one-hot compare/multiply/reduce on DVE, split in
   small pieces interleaved between ticks so they never stall the scan.
"""
import numpy as np

TAGSET = 32
START = 30
STOP = 31
B = 512
S = 2048
NCORES = 8
BC = B // NCORES          # 64 sequences per core
HALF = S // 2             # 1024 ticks per direction
CH = 64                   # emission steps per streamed chunk
NCH = HALF // CH          # 16 chunks per direction
GRP = 8                   # ticks per transpose/exp group (one PSUM bank)
MU = np.float32(4.3226)   # mean log-growth per step (measured offline)
REN = 128                 # renorm period in ticks

_CACHE = {}


def _build_nc(debug=False, gold=2, reps=1):
    import concourse.bacc as bacc
    import concourse.bass as bass
    import concourse.tile as tile
    from concourse import mybir

    f32 = mybir.dt.float32
    bf16 = mybir.dt.bfloat16
    i32 = mybir.dt.int32
    AF = mybir.ActivationFunctionType
    OP = mybir.AluOpType
    AX = mybir.AxisListType

    nc = bacc.Bacc("TRN2", target_bir_lowering=False, debug=False,
                   num_devices=NCORES)

    em_d = nc.dram_tensor("emissions", [BC, S, TAGSET], f32,
                          kind="ExternalInput").ap()
    tg_d = nc.dram_tensor("tags", [BC, S], i32, kind="ExternalInput").ap()
    tr_d = nc.dram_tensor("transitions", [TAGSET, TAGSET], f32,
                          kind="ExternalInput").ap()
    nll_d = nc.dram_tensor("nll", [1, BC], f32, kind="ExternalOutput").ap()

    with tile.TileContext(nc) as tc:
        with (
            tc.tile_pool(name="const", bufs=1) as cp,
            tc.tile_pool(name="chunk", bufs=3) as ccp,
            tc.tile_pool(name="oh", bufs=2) as ohp,
            tc.tile_pool(name="xt", bufs=2) as xtp,
            tc.tile_pool(name="state", bufs=3) as stp,
            tc.tile_pool(name="small", bufs=2) as smp,
            tc.tile_pool(name="trp", bufs=2, space="PSUM") as trp,
            tc.tile_pool(name="mmp", bufs=2, space="PSUM") as mmp,
            tc.tile_pool(name="finp", bufs=1, space="PSUM") as fip,
        ):
            # Optional on-device repetition (used only by test.py timing —
            # slope over reps cancels host/tunnel dispatch overhead).
            _loop = None
            if reps > 1:
                _loop = tc.For_i(
                    0, reps, 1,
                    hint_engines=(mybir.EngineType.PE, mybir.EngineType.DVE,
                                  mybir.EngineType.Activation))
                _loop.__enter__()

            # ---------------- setup: weights, identity, ones ----------------
            w = cp.tile([64, 64], f32)
            nc.vector.memset(w[:], 0.0)
            # fwd block: w[p, t] = trans[t, p]  (strided transpose DMA, tiny)
            nc.sync.dma_start(w[0:32, 0:32], tr_d.rearrange("a b -> b a"))
            # bwd block: w[32+p, 32+t] = trans[p, t]
            nc.sync.dma_start(w[32:64, 32:64], tr_d)
            # clamp -1e4 entries so the exp LUT stays in-range, then exp
            nc.vector.tensor_scalar_max(w[:], w[:], -80.0)
            nc.scalar.activation(w[:], w[:], AF.Exp)
            # zero the off-diagonal blocks again (exp(0)=1 crept in)
            nc.vector.memset(w[0:32, 32:64], 0.0)
            nc.vector.memset(w[32:64, 0:32], 0.0)
            # bf16 weights: bf16 matmul streams 1 col/cycle (fp32 is 4)
            wb16 = cp.tile([64, 64], bf16)
            nc.vector.tensor_copy(wb16[:], w[:])

            ones_t = cp.tile([64, 64], f32)
            nc.vector.memset(ones_t[:], 1.0)
            ones_b = cp.tile([64, 64], bf16)
            nc.vector.memset(ones_b[:], 1.0)
            negmu = cp.tile([64, 1], f32)
            nc.vector.memset(negmu[:], -float(MU))
            ident = cp.tile([64, 64], f32)
            nc.gpsimd.affine_select(
                out=ident[:], in_=ones_t[:], pattern=[[-1, 64]],
                compare_op=OP.is_equal, fill=0.0, base=0, channel_multiplier=1)

            # ---------------- gold emission score (one-hot) ----------------
            # e_score[b] = sum_s em[b, s, tags[b, s]]  computed in GRP-step
            # pieces as sum((t-iota == tag) * em) with DVE is_equal +
            # mult + reduce. The transition part of the gold score is tiny
            # and is folded in on the host during unshard.
            tags_sb = cp.tile([BC, S], i32)
            nc.sync.dma_start(tags_sb[:], tg_d)
            iota_t = cp.tile([BC, GRP * TAGSET], i32)
            nc.gpsimd.iota(iota_t[:], pattern=[[0, GRP], [1, TAGSET]], base=0,
                           channel_multiplier=0)
            NACC = NCH * (CH // GRP)  # one accum column per piece
            acc_e = cp.tile([BC, NACC], f32)
            nc.vector.memset(acc_e[:], 0.0)

            # ---------------- scan state init ----------------
            offacc = cp.tile([64, 64], f32)
            nc.vector.memset(offacc[:], 0.0)

            state = stp.tile([64, 64], bf16, tag="state")
            # one-hot inits: fwd rows = e_START, bwd rows = e_STOP
            nc.gpsimd.affine_select(
                out=state[0:32, :], in_=ones_b[0:32, :], pattern=[[0, 64]],
                compare_op=OP.is_equal, fill=0.0, base=-START,
                channel_multiplier=1)
            nc.gpsimd.affine_select(
                out=state[32:64, :], in_=ones_b[32:64, :], pattern=[[0, 64]],
                compare_op=OP.is_equal, fill=0.0, base=-STOP,
                channel_multiplier=1)

            # ---------------- chunk machinery ----------------
            # comb(g): [BC, CH*2*TAGSET] with interleaved layout per local
            # step l: cols [l*64, l*64+32) = emissions[:, g*CH+l, :] (fwd),
            # cols [l*64+32, l*64+64) = emissions[:, S-1-g*CH-l, :] (bwd).
            def load_chunk(g):
                comb = ccp.tile([BC, 2 * CH * TAGSET], f32, tag="comb")
                cv = comb[:].rearrange("b (s u t) -> b s u t",
                                       u=2, t=TAGSET)
                nc.sync.dma_start(cv[:, :, 0, :],
                                  em_d[:, g * CH:(g + 1) * CH, :])
                nc.sync.dma_start(
                    cv[:, :, 1, :],
                    em_d[:, S - 1 - g * CH:S - (g + 1) * CH - 1:-1, :])
                return comb

            # xt chunk: [64, CH*64] bf16; tick l occupies cols l*64:(l+1)*64
            # with fwd tags on partitions 0-31, bwd tags on 32-63.
            def prep_tick(comb, xt_chunk, l, trg):
                # one PE transpose per tick into the group PSUM tile
                j = l % GRP
                nc.tensor.transpose(
                    trg[:, j * 64:j * 64 + 64],
                    comb[:, l * 64:(l + 1) * 64], ident[:])
                if j == GRP - 1:
                    base = (l - j) * 64
                    nc.scalar.activation(
                        xt_chunk[:, base:base + GRP * 64], trg[:],
                        AF.Exp, bias=negmu[:])

            def gold_piece(comb, g, k):
                # one-hot gold for ticks [k*GRP, (k+1)*GRP) of chunk g,
                # both directions: 2 strided is_equal + 1 mult + 1 reduce
                iview = iota_t[:].rearrange("b (l t) -> b l t", t=TAGSET)
                oh = ohp.tile([BC, GRP * 2 * TAGSET], f32, tag="oh")
                ov = oh[:].rearrange("b (l u t) -> b l u t", u=2, t=TAGSET)
                for u in range(2):
                    if u == 0:
                        tsl = tags_sb[:, g * CH + k * GRP:
                                      g * CH + (k + 1) * GRP]
                    else:
                        hi = S - 1 - g * CH - k * GRP
                        tsl = tags_sb[:, hi:hi - GRP:-1]
                    tbc = tsl.rearrange("b l -> b l ()").to_broadcast(
                        [BC, GRP, TAGSET])
                    nc.vector.tensor_tensor(
                        out=ov[:, :, u, :], in0=iview, in1=tbc,
                        op=OP.is_equal)
                scrap = ohp.tile([BC, GRP * 2 * TAGSET], f32, tag="scrap")
                nc.vector.tensor_mul(scrap[:], oh[:],
                                     comb[:, k * GRP * 64:(k + 1) * GRP * 64])
                col = g * (CH // GRP) + k
                nc.vector.tensor_reduce(
                    acc_e[:, col:col + 1], scrap[:], axis=AX.X, op=OP.add)

            # ---------------- prologue: chunk 0 prep ----------------
            comb_cur = load_chunk(0)
            trg = None
            xtc_cur = xtp.tile([64, CH * 64], bf16, tag="xtc")
            for l in range(CH):
                if l % GRP == 0:
                    trg = trp.tile([64, GRP * 64], f32, tag="trg")
                prep_tick(comb_cur, xtc_cur, l, trg)

            # ---------------- main scan ----------------
            for g in range(NCH):
                if g + 1 < NCH:
                    comb_nxt = load_chunk(g + 1)
                    xtc_nxt = xtp.tile([64, CH * 64], bf16, tag="xtc")
                else:
                    comb_nxt = None
                    xtc_nxt = None
                for l in range(CH):
                    tau = g * CH + l
                    # pipeline: prep next chunk's xt, one tick per tick
                    if comb_nxt is not None:
                        if l % GRP == 0:
                            trg = trp.tile([64, GRP * 64], f32, tag="trg")
                        prep_tick(comb_nxt, xtc_nxt, l, trg)
                    # gold pieces for the current chunk, spread out
                    if gold >= 2 and l % GRP == 3:
                        gold_piece(comb_cur, g, l // GRP)

                    ps = mmp.tile([64, 64], f32, tag="mm")
                    nc.tensor.matmul(ps[:], wb16[:], state[:], start=True,
                                     stop=True)
                    nstate = stp.tile([64, 64], bf16, tag="state")
                    nc.vector.tensor_mul(nstate[:], ps[:],
                                         xtc_cur[:, l * 64:(l + 1) * 64])
                    state = nstate

                    if (tau + 1) % REN == 0:
                        # proxy rows to f32 first (reciprocal/Ln need f32 in)
                        pr = smp.tile([64, 64], f32, tag="pr")
                        nc.vector.tensor_copy(pr[0:1, :], state[0:1, :])
                        nc.vector.tensor_copy(pr[32:33, :], state[32:33, :])
                        rec = smp.tile([64, 64], f32, tag="rec")
                        nc.vector.reciprocal(rec[0:1, :], pr[0:1, :])
                        nc.vector.reciprocal(rec[32:33, :], pr[32:33, :])
                        bc_ps = fip.tile([64, 64], f32, tag="bc")
                        nc.tensor.matmul(bc_ps[0:32, :], ones_t[0:1, 0:32],
                                         rec[0:1, :], start=True, stop=True)
                        nc.tensor.matmul(bc_ps[32:64, :], ones_t[32:33, 0:32],
                                         rec[32:33, :], start=True, stop=True,
                                         tile_position=(32, 32))
                        lg = smp.tile([64, 64], f32, tag="lg")
                        nc.scalar.activation(lg[0:1, :], pr[0:1, :], AF.Ln)
                        nc.scalar.activation(lg[32:33, :], pr[32:33, :],
                                             AF.Ln)
                        nc.vector.tensor_add(offacc[0:1, :], offacc[0:1, :],
                                             lg[0:1, :])
                        nc.vector.tensor_add(offacc[32:33, :],
                                             offacc[32:33, :], lg[32:33, :])
                        rstate = stp.tile([64, 64], bf16, tag="state")
                        nc.vector.tensor_mul(rstate[:], state[:], bc_ps[:])
                        state = rstate
                comb_cur = comb_nxt
                xtc_cur = xtc_nxt

            # ---------------- finale ----------------
            # beta_1023 = M^T gamma_1024: bwd-final matmul with weights
            # placed so the output lands on partitions 0-31 (aligned with
            # the fwd state for the elementwise dot).
            wb = cp.tile([64, 64], bf16)
            nc.vector.memset(wb[:], 0.0)
            nc.sync.dma_start(wb[32:64, 0:32], wb16[32:64, 32:64])
            psf = mmp.tile([64, 64], f32, tag="mm")
            nc.tensor.matmul(psf[0:32, :], wb[32:64, 0:32], state[32:64, :],
                             start=True, stop=True)
            zp = smp.tile([64, 64], f32, tag="zp")
            nc.vector.tensor_mul(zp[0:32, :], psf[0:32, :], state[0:32, :])
            zsum = fip.tile([1, 64], f32, tag="zsum")
            nc.tensor.matmul(zsum[0:1, :], ones_t[0:32, 0:1], zp[0:32, :],
                             start=True, stop=True)
            gold_c = cp.tile([BC, 1], f32)
            nc.vector.tensor_reduce(gold_c[:], acc_e[:], axis=AX.X, op=OP.add)
            lz = smp.tile([64, 64], f32, tag="lz")
            nc.scalar.activation(lz[0:1, :], zsum[0:1, :], AF.Ln)
            ob = smp.tile([64, 64], f32, tag="ob")
            nc.sync.dma_start(ob[0:1, :], offacc[32:33, :])
            nc.vector.tensor_add(lz[0:1, :], lz[0:1, :], offacc[0:1, :])
            nc.vector.tensor_add(lz[0:1, :], lz[0:1, :], ob[0:1, :])
            # logZ = lz + MU*S;   nll = logZ - gold
            goldT = fip.tile([1, 64], f32, tag="goldT")
            nc.tensor.transpose(goldT[0:1, :], gold_c[:, 0:1], ident[:])
            nc.vector.tensor_sub(lz[0:1, :], lz[0:1, :], goldT[0:1, :])
            nc.vector.tensor_scalar_add(lz[0:1, :], lz[0:1, :],
                                        float(MU) * S)
            nc.sync.dma_start(nll_d, lz[0:1, :])

            if _loop is not None:
                _loop.__exit__(None, None, None)

    nc.compile()
    return nc


def _get_nc():
    if "nc" not in _CACHE:
        _CACHE["nc"] = _build_nc()
    return _CACHE["nc"]


def kernel(emissions, transitions, tags):
    from concourse.bass_utils import run_bass_kernel_spmd

    em = np.ascontiguousarray(np.asarray(emissions, dtype=np.float32))
    tr = np.ascontiguousarray(np.asarray(transitions, dtype=np.float32))
    tg = np.ascontiguousarray(np.asarray(tags, dtype=np.int32))

    nc = _get_nc()
    in_maps = [
        {
            "emissions": em[c * BC:(c + 1) * BC],
            "tags": tg[c * BC:(c + 1) * BC],
            "transitions": tr,
        }
        for c in range(NCORES)
    ]
    res = run_bass_kernel_spmd(nc, in_maps, list(range(NCORES)))
    nll = np.concatenate([res.results[c]["nll"][0] for c in range(NCORES)])
    t_sc = (tr[tg[:, 1:], tg[:, :-1]].sum(axis=1)
            + tr[tg[:, 0], START] + tr[STOP, tg[:, -1]])
    total = np.sum(nll.astype(np.float64)) - np.sum(t_sc.astype(np.float64))
    return np.array(total, dtype=np.float32)
